# revision 15
# baseline (speedup 1.0000x reference)
"""GAT attention head (gnn_message_passing) on 8 TRN2 NeuronCores.

v5 design (a_r-prescaled table, PE one-hot f1 lookup, row-sum f2):
  - Nodes partitioned across 8 cores; per core, dest nodes are LPT-balanced
    into NB blocks of 128 slots.
  - Host folds a_r into the weights (W' = W * a_r[None, :], wl = W @ a_l),
    so the node matmul rhs [128, 129] yields seqp = feat@W' plus an f1
    column in one pass.  Table rows are seqp in bf16 (256 B), AllGathered
    per region into shared tables.  The final output is rescaled by 1/a_r
    (exact in floating point).
  - Per-edge f2 = seq@a_r = row-sum of the gathered row: a single-source
    reduce, split between DVE (tensor_reduce) and the idle ACT engine
    (activation Copy + accum_out).
  - Per-edge f1 lookup runs on the PE: a host-built transposed one-hot
    (fp8) per tile is lhsT against the f1 column, producing f1e as psum
    columns - no DVE one-hot dot.
  - wt = (iota==rowrel)*w in one fused DVE op per tile; numerator matmul
    wt.T @ seqp and denominator wt.T @ ones accumulate into one psum tile
    per dest block.
"""

import math
import sys

import numpy as np

for _p in ("/opt/trn_rl_repo",):
    if _p not in sys.path:
        sys.path.insert(0, _p)

import concourse.bacc as bacc
import concourse.bass as bass
import concourse.mybir as mybir
import concourse.tile as tile
from concourse.bass_utils import run_bass_kernel_spmd

F32 = mybir.dt.float32
BF16 = mybir.dt.bfloat16
FP8 = mybir.dt.float8e4
U8 = mybir.dt.uint8
I16 = mybir.dt.int16
AF = mybir.ActivationFunctionType
ALU = mybir.AluOpType

GCHUNK = 8           # tiles per dma_gather call (1024-idx ucode cap)
ACT_F2_FRAC = 0.3    # fraction of per-tile f2 reduces on the ACT engine


class _Cfg:
    def __init__(self, N, E, IN, OUT, C, sup_blocks=8, regions=4):
        assert N % C == 0
        self.N, self.E, self.IN, self.OUT, self.C = N, E, IN, OUT, C
        self.KI = IN // 128
        assert IN == self.KI * 128
        assert OUT == 128, "builder assumes OUT==128"
        self.NPC = N // C                     # nodes per core
        self.REG = regions
        self.QN = math.ceil(self.NPC / regions)   # nodes per quarter
        per_reg = math.ceil(self.QN / 128) + 1
        self.NB = per_reg * regions           # 104 for NPC=12500, REG=4
        self.BPQ = per_reg                    # blocks per quarter
        self.NSLOT = self.NB * 128
        self.RSLOT = self.NSLOT // regions    # slots per region per core
        self.RTB = self.RSLOT // 128          # node tiles per region
        self.RROWS = C * self.RSLOT           # region table rows
        assert self.RROWS <= 32767, "dma_gather int16 index range"
        self.SUP = sup_blocks
        self.supers = []
        b = 0
        while b < self.NB:
            nbl = min(sup_blocks, self.NB - b)
            self.supers.append((b, nbl))
            b += nbl
        self.meta = None


def _prep_host(cfg, feat, W, a_l, b_l, a_r, b_r, bias, row, col):
    C, NPC, NB, NSLOT = cfg.C, cfg.NPC, cfg.NB, cfg.NSLOT
    N, IN, OUT, REG, RSLOT = cfg.N, cfg.IN, cfg.OUT, cfg.REG, cfg.RSLOT

    row = row.astype(np.int64)
    col = col.astype(np.int64)
    core = row // NPC
    QN, BPQ = cfg.QN, cfg.BPQ

    # node -> original-id quarter (the source region)
    oq = np.minimum((np.arange(N) % NPC) // QN, REG - 1)

    # per-dest in-edge counts split by source region
    vreg = np.zeros((N, REG), np.int64)
    np.add.at(vreg, (row, oq[col]), 1)
    deg = vreg.sum(axis=1)

    # --- region-aware LPT: per (core, quarter), balance per-region loads
    # across the quarter's BPQ blocks (capacity 128 dests each) ------------
    newlocal = np.empty(N, np.int64)
    for c in range(C):
        for q in range(REG):
            n0 = c * NPC + q * QN
            n1 = min(c * NPC + NPC, n0 + QN)
            ids = np.arange(n0, n1)
            order = ids[np.argsort(-deg[n0:n1], kind="stable")]
            loads = np.zeros((BPQ, REG), np.int64)
            counts = np.zeros(BPQ, np.int64)
            vq = vreg[order]
            for i, dest in enumerate(order):
                cand = (loads + vq[i]).max(axis=1)
                cand[counts >= 128] = 1 << 50
                b = int(np.argmin(cand))
                newlocal[dest] = (q * BPQ + b) * 128 + counts[b]
                counts[b] += 1
                loads[b] += vq[i]

    # --- per-edge derived ids --------------------------------------------
    snl = newlocal[col]                       # source slot within its core
    ereg = oq[col]                            # source region (= slot quarter)
    erow = (col // NPC) * RSLOT + (snl - ereg * RSLOT)  # region-local row
    edslot = newlocal[row]                    # dest slot
    eblk = edslot // 128
    epos = (edslot % 128).astype(np.float32)

    # counts per (core, block, region); SPMD => pad to max over cores
    cnts = np.zeros((C, NB, REG), np.int64)
    np.add.at(cnts, (core, eblk, ereg), 1)
    runmax = cnts.max(axis=0)                 # [NB, REG]
    tiles_br = (runmax + 127) // 128          # tiles per (block, region)

    # --- tile layout ------------------------------------------------------
    meta = {"supers": []}
    gtile = 0
    t0_br = np.full((NB, REG), -1, np.int64)  # global first tile of run
    for (b0, nbl) in cfg.supers:
        sup = {"b0": b0, "nb": nbl, "gt0": gtile, "g_calls": [], "blocks": []}
        scol = 0
        for r in range(REG):
            nt_r = int(tiles_br[b0:b0 + nbl, r].sum())
            if nt_r:
                sup["g_calls"].append(
                    {"region": r, "tile0": scol, "ntiles": nt_r})
            for bi in range(nbl):
                t0_br[b0 + bi, r] = gtile + scol
                scol += int(tiles_br[b0 + bi, r])
        sup["ntiles"] = scol
        for bi in range(nbl):
            b = b0 + bi
            runs = [(int(t0_br[b, r]), int(tiles_br[b, r]))
                    for r in range(REG) if tiles_br[b, r] > 0]
            sup["blocks"].append({"b": b, "runs": runs})
        gtile += scol
        meta["supers"].append(sup)
    NTILES = gtile
    meta["NTILES"] = NTILES

    # --- per-core index arrays -------------------------------------------
    idxg = np.zeros((C, 128, NTILES * 8), np.int16)
    rowrel = np.full((C, 128, NTILES), -1.0, np.float32)

    okey = (core * NB + eblk) * REG + ereg
    oorder = np.argsort(okey, kind="stable")
    ks = okey[oorder]
    starts = np.searchsorted(ks, np.arange(C * NB * REG))
    slot_in_run = np.empty(cfg.E, np.int64)
    slot_in_run[oorder] = np.arange(cfg.E) - starts[ks]

    gt0_e = t0_br[eblk, ereg]                 # first global tile of the run
    ek = gt0_e * 128 + slot_in_run            # global slot id
    etile = ek // 128
    epart = ek % 128

    oht = np.zeros((C, 128, NTILES * 128), np.uint8)
    ONE_FP8 = 0x38          # 1.0 in float8_e4m3
    for c in range(C):
        m = core == c
        rowrel[c, epart[m], etile[m]] = epos[m]
        kk = ek[m]
        idxg[c, kk % 16, (kk // 16)] = erow[m].astype(np.int16)
        oht[c, epos[m].astype(np.int64),
            etile[m] * 128 + epart[m]] = ONE_FP8
    for g in range(1, 8):
        idxg[:, g * 16:(g + 1) * 16, :] = idxg[:, 0:16, :]

    # --- parameters -------------------------------------------------------
    import ml_dtypes

    inv = np.empty((C, NSLOT), np.int64)
    have = np.zeros((C, NSLOT), bool)
    for c in range(C):
        nl = newlocal[c * NPC:(c + 1) * NPC]
        inv[c, nl] = np.arange(NPC)
        have[c, nl] = True
    featT = np.zeros((C, IN, NSLOT), ml_dtypes.bfloat16)
    for c in range(C):
        idx = inv[c][have[c]]
        featT[c][:, have[c]] = feat[c * NPC + idx].T.astype(ml_dtypes.bfloat16)

    a_r64 = np.asarray(a_r, np.float64)
    a_rs = np.where(np.abs(a_r64) < 1e-30, 1e-30, a_r64)
    Wp = (np.asarray(W, np.float64) * a_rs[None, :])        # [IN, OUT]
    wl = np.asarray(W, np.float64) @ np.asarray(a_l, np.float64)  # [IN]
    rhsk = []
    for k in range(cfg.KI):
        blk = np.concatenate(
            [Wp[k * 128:(k + 1) * 128],
             wl[k * 128:(k + 1) * 128, None]], axis=1)       # [128, 129]
        rhsk.append(np.ascontiguousarray(blk).astype(ml_dtypes.bfloat16))
    invarb = np.tile((1.0 / a_rs).astype(np.float32)[None, :], (128, 1))
    biasb = np.tile(np.asarray(bias, np.float32)[None, :], (128, 1))
    bsum = float(np.asarray(b_l, np.float64) + np.asarray(b_r, np.float64))
    bs_col = np.full((128, 1), bsum, np.float32)
    iota = np.tile(np.arange(128, dtype=np.float32)[None, :], (128, 1))
    iotab = iota.astype(ml_dtypes.bfloat16)

    in_maps = []
    for c in range(C):
        m = {
            "featT": featT[c], "biasb": biasb, "invarb": invarb,
            "bs": bs_col, "iotab": iotab,
            "idxg": idxg[c], "rowrel": rowrel[c],
            "oht": oht[c].view(ml_dtypes.float8_e4m3),
        }
        for k in range(cfg.KI):
            m[f"wk{k}"] = rhsk[k]
        in_maps.append(m)

    cfg.meta = meta

    def assemble(outs):
        full = np.empty((N, OUT), np.float32)
        for c in range(C):
            o = outs[c]["out"]
            nlc = newlocal[c * NPC:(c + 1) * NPC]
            full[c * NPC:(c + 1) * NPC] = o[nlc]
        return full

    return in_maps, assemble


def _build_program(cfg):
    C, IN, OUT = cfg.C, cfg.IN, cfg.OUT
    NB, NSLOT, KI, REG = cfg.NB, cfg.NSLOT, cfg.KI, cfg.REG
    RSLOT, RTB, RROWS = cfg.RSLOT, cfg.RTB, cfg.RROWS
    meta = cfg.meta
    NTILES = meta["NTILES"]
    NTMAX = max(s["ntiles"] for s in meta["supers"])
    FTC = RTB // 2 if RTB % 2 == 0 else RTB     # node tiles per featT chunk
    FCH = FTC * 128                             # featT load chunk columns

    nc = bacc.Bacc(None, num_swdge_queues=4, dynamic_dma_scratch_size=32768)
    featT = nc.declare_dram_parameter("featT", [IN, NSLOT], BF16, isOutput=False)
    wk = [nc.declare_dram_parameter(f"wk{k}", [128, OUT + 1], BF16,
                                    isOutput=False) for k in range(KI)]
    biasb = nc.declare_dram_parameter("biasb", [128, OUT], F32, isOutput=False)
    invarb = nc.declare_dram_parameter("invarb", [128, OUT], F32,
                                       isOutput=False)
    bs = nc.declare_dram_parameter("bs", [128, 1], F32, isOutput=False)
    iotab = nc.declare_dram_parameter("iotab", [128, 128], BF16, isOutput=False)
    idxg = nc.declare_dram_parameter("idxg", [128, NTILES * 8], I16,
                                     isOutput=False)
    rowrel = nc.declare_dram_parameter("rowrel", [128, NTILES], F32,
                                       isOutput=False)
    ohtp = nc.declare_dram_parameter("oht", [128, NTILES * 128], FP8,
                                     isOutput=False)
    outp = nc.declare_dram_parameter("out", [NSLOT, OUT], F32, isOutput=True)

    with tile.TileContext(nc) as tc:
        with (
            tc.tile_pool(name="dram", bufs=1, space="DRAM") as dram,
            tc.tile_pool(name="consts", bufs=1) as cp,
            tc.tile_pool(name="nfeat", bufs=2) as nfp,
            tc.tile_pool(name="naug", bufs=2) as nap,
            tc.tile_pool(name="npsum", bufs=2, space="PSUM") as npp,
            tc.tile_pool(name="eidx", bufs=2) as eip,
            tc.tile_pool(name="eoht", bufs=2) as ehp,
            tc.tile_pool(name="f1ps", bufs=2, space="PSUM") as f1pp,
            tc.tile_pool(name="egath", bufs=2) as egp,
            tc.tile_pool(name="ecol", bufs=2) as ecp,
            tc.tile_pool(name="ewt", bufs=4) as ewp,
            tc.tile_pool(name="epsum", bufs=2, space="PSUM") as epp,
            tc.tile_pool(name="eout", bufs=3) as eop,
        ):
            agin = [dram.tile([RSLOT, OUT], BF16, name=f"agin{r}")
                    for r in range(REG)]
            tabr = [dram.tile([RROWS, OUT], BF16, name=f"tabr{r}",
                              addr_space="Shared") for r in range(REG)]

            # ---- constants ----
            wk_sb = []
            for k in range(KI):
                w_t = cp.tile([128, OUT + 1], BF16, name=f"wksb{k}")
                nc.sync.dma_start(w_t[:], wk[k][:])
                wk_sb.append(w_t)
            biasb_sb = cp.tile([128, OUT], F32)
            nc.sync.dma_start(biasb_sb[:], biasb[:])
            invarb_sb = cp.tile([128, OUT], F32)
            nc.sync.dma_start(invarb_sb[:], invarb[:])
            bs_sb = cp.tile([128, 1], F32)
            nc.sync.dma_start(bs_sb[:], bs[:])
            iota_sb = cp.tile([128, 128], BF16)
            nc.sync.dma_start(iota_sb[:], iotab[:])
            ones_sb = cp.tile([128, 1], BF16)
            nc.vector.memset(ones_sb[:], 1.0)
            f1acc = cp.tile([128, 128], F32)
            nc.vector.memset(f1acc[:], 0.0)
            f1bbq = []

            # ---- node phase (region-major) ----
            for r in range(REG):
                fts = {}
                for k in range(KI):
                    for h in range(RTB // FTC):
                        ft = nfp.tile([128, FCH], BF16, name=f"ft{k}{h}")
                        c0 = r * RSLOT + h * FCH
                        nc.sync.dma_start(
                            ft[:], featT[k * 128:(k + 1) * 128, c0:c0 + FCH])
                        fts[(k, h)] = ft
                aug = nap.tile([128, RSLOT], BF16, name="aug")
                for ntl in range(RTB):
                    nt = r * RTB + ntl
                    ps = npp.tile([128, OUT + 1], F32)
                    for k in range(KI):
                        h, off = divmod(ntl * 128, FCH)
                        ft = fts[(k, h)]
                        nc.tensor.matmul(ps[:], lhsT=ft[:, off:off + 128],
                                         rhs=wk_sb[k][:],
                                         start=(k == 0), stop=(k == KI - 1))
                    nc.vector.tensor_copy(aug[:, ntl * 128:(ntl + 1) * 128],
                                          ps[:, 0:128])
                    nc.vector.tensor_copy(f1acc[:, nt:nt + 1],
                                          ps[:, 128:129])
                f1q = cp.tile([128, RTB], BF16, name=f"f1bbq{r}")
                nc.vector.tensor_scalar(
                    out=f1q[:], in0=f1acc[:, r * RTB:(r + 1) * RTB],
                    scalar1=bs_sb[:], scalar2=None, op0=ALU.add)
                f1bbq.append(f1q)
                nc.sync.dma_start(
                    agin[r][:, :].rearrange("(t p) o -> p t o", p=128),
                    aug[:].rearrange("p (t o) -> p t o", o=OUT))
                nc.gpsimd.collective_compute(
                    "AllGather", ALU.bypass,
                    replica_groups=[list(range(C))],
                    ins=[agin[r].opt()],
                    outs=[tabr[r].opt()],
                )


            # ---- edge phase ----
            self_qn = [0]
            tix = [0]     # global tile counter for DVE/ACT f2 split
            for sup in meta["supers"]:
                ntiles = sup["ntiles"]
                gt0 = sup["gt0"]
                ixg = eip.tile([128, NTMAX * 8], I16, name="ixg")
                nc.sync.dma_start(ixg[:, 0:ntiles * 8],
                                  idxg[:, gt0 * 8:(gt0 + ntiles) * 8])
                rr_sb = eip.tile([128, NTMAX], F32, name="rr_sb")
                nc.sync.dma_start(rr_sb[:, 0:ntiles],
                                  rowrel[:, gt0:gt0 + ntiles])
                oht_sb = ehp.tile([128, NTMAX * 128], FP8, name="oht_sb")
                nc.sync.dma_start(oht_sb[:, 0:ntiles * 128],
                                  ohtp[:, gt0 * 128:(gt0 + ntiles) * 128])

                G = egp.tile([128, NTMAX * 128], BF16, name="G")
                G3 = G[:].rearrange("p (t c) -> p t c", c=128)
                for g in sup["g_calls"]:
                    r = g["region"]
                    for ct0 in range(0, g["ntiles"], GCHUNK):
                        cn = min(GCHUNK, g["ntiles"] - ct0)
                        lt0 = g["tile0"] + ct0
                        nc.gpsimd.dma_gather(
                            out_ap=G[:, lt0 * 128:(lt0 + cn) * 128]
                            .rearrange("p (t e) -> p t e", e=128),
                            in_ap=tabr[r][:],
                            idxs_ap=ixg[:, lt0 * 8:(lt0 + cn) * 8],
                            num_idxs=cn * 128,
                            num_idxs_reg=cn * 128,
                            elem_size=128,
                            queue_num=self_qn[0] % 4,
                        )
                        self_qn[0] += 1

                # f2 per edge = row-sum of the gathered (a_r-scaled) row;
                # split between DVE reduce and ACT copy-accum
                F2 = ecp.tile([128, NTMAX], F32, name="F2")
                for t in range(ntiles):
                    tix[0] += 1
                    if (tix[0] % 10) < int(ACT_F2_FRAC * 10):
                        dmy = ecp.tile([128, 128], BF16, name="dmy")
                        nc.scalar.activation(dmy[:], G3[:, t, :], AF.Copy,
                                             accum_out=F2[:, t:t + 1])
                    else:
                        nc.vector.tensor_reduce(
                            out=F2[:, t:t + 1], in_=G3[:, t, :],
                            axis=mybir.AxisListType.X, op=ALU.add)

                # f1e per edge on PE: psf[:, j] = OHT_t^T @ f1col(b)
                TT = ecp.tile([128, NTMAX], F32, name="TT")
                BPQ = cfg.BPQ
                for blk in sup["blocks"]:
                    b = blk["b"]
                    q = b // BPQ
                    psf = f1pp.tile([128, 32], F32, name="psf")
                    j = 0
                    spans = []
                    for (t0, nt) in blk["runs"]:
                        spans.append((j, t0 - gt0, nt))
                        for t in range(t0 - gt0, t0 - gt0 + nt):
                            nc.tensor.matmul(
                                psf[:, j:j + 1],
                                lhsT=oht_sb[:, t * 128:(t + 1) * 128],
                                rhs=f1bbq[q][:, (b % BPQ):(b % BPQ) + 1],
                                start=True, stop=True)
                            j += 1
                    # t = f1e + f2 written straight into the super's TT cols
                    for (j0, tl0, nt) in spans:
                        nc.vector.tensor_tensor(
                            out=TT[:, tl0:tl0 + nt],
                            in0=psf[:, j0:j0 + nt],
                            in1=F2[:, tl0:tl0 + nt], op=ALU.add)

                # w = exp(0.6 t + 0.4 |t|)
                AA = ecp.tile([128, NTMAX], F32, name="AA")
                nc.scalar.activation(AA[:, 0:ntiles], TT[:, 0:ntiles],
                                     AF.Abs, scale=0.4)
                ZZ = ecp.tile([128, NTMAX], F32, name="ZZ")
                nc.vector.scalar_tensor_tensor(
                    out=ZZ[:, 0:ntiles], in0=TT[:, 0:ntiles], scalar=0.6,
                    in1=AA[:, 0:ntiles], op0=ALU.mult, op1=ALU.add)
                WW = ecp.tile([128, NTMAX], F32, name="WW")
                nc.scalar.activation(WW[:, 0:ntiles], ZZ[:, 0:ntiles], AF.Exp)

                for blk in sup["blocks"]:
                    b = blk["b"]
                    tl = []
                    for (t0, nt) in blk["runs"]:
                        tl.extend(range(t0 - gt0, t0 - gt0 + nt))
                    if not tl:
                        continue
                    ps = epp.tile([128, OUT], F32, name="bps")
                    ps2 = epp.tile([128, 1], F32, name="bps2")
                    for j, t in enumerate(tl):
                        wt = ewp.tile([128, 128], BF16, name="wt")
                        nc.vector.tensor_scalar(
                            out=wt[:], in0=iota_sb[:],
                            scalar1=rr_sb[:, t:t + 1],
                            scalar2=WW[:, t:t + 1],
                            op0=ALU.is_equal, op1=ALU.mult)
                        first = j == 0
                        last = j == len(tl) - 1
                        nc.tensor.matmul(
                            ps[:], lhsT=wt[:],
                            rhs=G3[:, t, :],
                            start=first, stop=last)
                        nc.tensor.matmul(
                            ps2[:], lhsT=wt[:],
                            rhs=ones_sb[:],
                            start=first, stop=last)
                    sden = eop.tile([128, 1], F32, name="sden")
                    nc.vector.tensor_scalar(out=sden[:],
                                            in0=ps2[:],
                                            scalar1=1e-9, scalar2=None,
                                            op0=ALU.add)
                    rcp = eop.tile([128, 1], F32, name="rcp")
                    nc.vector.reciprocal(rcp[:], sden[:])
                    xx1 = eop.tile([128, OUT], F32, name="xx1")
                    nc.vector.scalar_tensor_tensor(
                        out=xx1[:], in0=ps[:], scalar=rcp[:],
                        in1=invarb_sb[:], op0=ALU.mult, op1=ALU.mult)
                    xx = eop.tile([128, OUT], F32, name="xx")
                    nc.vector.tensor_tensor(out=xx[:], in0=xx1[:],
                                            in1=biasb_sb[:], op=ALU.add)
                    ee = eop.tile([128, OUT], F32, name="ee")
                    nc.scalar.activation(ee[:], xx[:], AF.Exp)
                    ov = eop.tile([128, OUT], F32, name="ov")
                    nc.vector.tensor_scalar(out=ov[:], in0=ee[:],
                                            scalar1=-1.0, scalar2=None,
                                            op0=ALU.add)
                    mk = eop.tile([128, OUT], U8, name="mk")
                    nc.vector.tensor_scalar(out=mk[:], in0=xx[:],
                                            scalar1=0.0, scalar2=None,
                                            op0=ALU.is_gt)
                    nc.vector.copy_predicated(ov[:], mk[:], xx[:])
                    nc.sync.dma_start(outp[b * 128:(b + 1) * 128, :], ov[:])

    nc.finalize()
    return nc


def _run(cfg, inputs, trace=False):
    in_maps, assemble = _prep_host(
        cfg,
        np.asarray(inputs["feat"], np.float32),
        np.asarray(inputs["W"], np.float32),
        np.asarray(inputs["a_l"], np.float32),
        np.asarray(inputs["b_l"], np.float32),
        np.asarray(inputs["a_r"], np.float32),
        np.asarray(inputs["b_r"], np.float32),
        np.asarray(inputs["bias"], np.float32),
        np.asarray(inputs["row"]),
        np.asarray(inputs["col"]),
    )
    nc = _build_program(cfg)
    res = run_bass_kernel_spmd(nc, in_maps, list(range(cfg.C)), trace=trace)
    return assemble(res.results), res


def kernel(**inputs):
    feat = np.asarray(inputs["feat"])
    row = np.asarray(inputs["row"])
    cfg = _Cfg(N=feat.shape[0], E=row.shape[0], IN=feat.shape[1],
               OUT=np.asarray(inputs["W"]).shape[1], C=8)
    out, _ = _run(cfg, inputs, trace=False)
    return out


# revision 16
# speedup vs baseline: 1.2900x; 1.2900x over previous
"""GAT attention head (gnn_message_passing) on 8 TRN2 NeuronCores.

v5 design (a_r-prescaled table, PE one-hot f1 lookup, row-sum f2):
  - Nodes partitioned across 8 cores; per core, dest nodes are LPT-balanced
    into NB blocks of 128 slots.
  - Host folds a_r into the weights (W' = W * a_r[None, :], wl = W @ a_l),
    so the node matmul rhs [128, 129] yields seqp = feat@W' plus an f1
    column in one pass.  Table rows are seqp in bf16 (256 B), AllGathered
    per region into shared tables.  The final output is rescaled by 1/a_r
    (exact in floating point).
  - Per-edge f2 = seq@a_r = row-sum of the gathered row: a single-source
    reduce, split between DVE (tensor_reduce) and the idle ACT engine
    (activation Copy + accum_out).
  - Per-edge f1 lookup runs on the PE: a host-built transposed one-hot
    (fp8) per tile is lhsT against the f1 column, producing f1e as psum
    columns - no DVE one-hot dot.
  - wt = (iota==rowrel)*w in one fused DVE op per tile; numerator matmul
    wt.T @ seqp and denominator wt.T @ ones accumulate into one psum tile
    per dest block.
"""

import math
import sys

import numpy as np

for _p in ("/opt/trn_rl_repo",):
    if _p not in sys.path:
        sys.path.insert(0, _p)

import concourse.bacc as bacc
import concourse.bass as bass
import concourse.mybir as mybir
import concourse.tile as tile
from concourse.bass_utils import run_bass_kernel_spmd

F32 = mybir.dt.float32
BF16 = mybir.dt.bfloat16
FP8 = mybir.dt.float8e4
U8 = mybir.dt.uint8
I16 = mybir.dt.int16
AF = mybir.ActivationFunctionType
ALU = mybir.AluOpType

GCHUNK = 8           # tiles per dma_gather call (1024-idx ucode cap)
ACT_F2_FRAC = 0.3    # fraction of per-tile f2 reduces on the ACT engine


class _Cfg:
    def __init__(self, N, E, IN, OUT, C, sup_blocks=6, regions=4):
        assert N % C == 0
        self.N, self.E, self.IN, self.OUT, self.C = N, E, IN, OUT, C
        self.KI = IN // 128
        assert IN == self.KI * 128
        assert OUT == 128, "builder assumes OUT==128"
        self.NPC = N // C                     # nodes per core
        self.REG = regions
        self.QN = math.ceil(self.NPC / regions)   # nodes per quarter
        per_reg = math.ceil(self.QN / 128) + 1
        self.NB = per_reg * regions           # 104 for NPC=12500, REG=4
        self.BPQ = per_reg                    # blocks per quarter
        self.NSLOT = self.NB * 128
        self.RSLOT = self.NSLOT // regions    # slots per region per core
        self.RTB = self.RSLOT // 128          # node tiles per region
        self.RROWS = C * self.RSLOT           # region table rows
        assert self.RROWS <= 32767, "dma_gather int16 index range"
        self.SUP = sup_blocks
        self.supers = []
        b = 0
        while b < self.NB:
            nbl = min(sup_blocks, self.NB - b)
            self.supers.append((b, nbl))
            b += nbl
        self.meta = None


def _prep_host(cfg, feat, W, a_l, b_l, a_r, b_r, bias, row, col):
    C, NPC, NB, NSLOT = cfg.C, cfg.NPC, cfg.NB, cfg.NSLOT
    N, IN, OUT, REG, RSLOT = cfg.N, cfg.IN, cfg.OUT, cfg.REG, cfg.RSLOT

    row = row.astype(np.int64)
    col = col.astype(np.int64)
    core = row // NPC
    QN, BPQ = cfg.QN, cfg.BPQ

    # node -> original-id quarter (the source region)
    oq = np.minimum((np.arange(N) % NPC) // QN, REG - 1)

    # per-dest in-edge counts split by source region
    vreg = np.zeros((N, REG), np.int64)
    np.add.at(vreg, (row, oq[col]), 1)
    deg = vreg.sum(axis=1)

    # --- region-aware LPT: per (core, quarter), balance per-region loads
    # across the quarter's BPQ blocks (capacity 128 dests each) ------------
    newlocal = np.empty(N, np.int64)
    for c in range(C):
        for q in range(REG):
            n0 = c * NPC + q * QN
            n1 = min(c * NPC + NPC, n0 + QN)
            ids = np.arange(n0, n1)
            order = ids[np.argsort(-deg[n0:n1], kind="stable")]
            loads = np.zeros((BPQ, REG), np.int64)
            counts = np.zeros(BPQ, np.int64)
            vq = vreg[order]
            for i, dest in enumerate(order):
                cand = (loads + vq[i]).max(axis=1)
                cand[counts >= 128] = 1 << 50
                b = int(np.argmin(cand))
                newlocal[dest] = (q * BPQ + b) * 128 + counts[b]
                counts[b] += 1
                loads[b] += vq[i]

    # --- per-edge derived ids --------------------------------------------
    snl = newlocal[col]                       # source slot within its core
    ereg = oq[col]                            # source region (= slot quarter)
    erow = (col // NPC) * RSLOT + (snl - ereg * RSLOT)  # region-local row
    edslot = newlocal[row]                    # dest slot
    eblk = edslot // 128
    epos = (edslot % 128).astype(np.float32)

    # counts per (core, block, region); SPMD => pad to max over cores
    cnts = np.zeros((C, NB, REG), np.int64)
    np.add.at(cnts, (core, eblk, ereg), 1)
    runmax = cnts.max(axis=0)                 # [NB, REG]
    tiles_br = (runmax + 127) // 128          # tiles per (block, region)

    # --- tile layout ------------------------------------------------------
    meta = {"supers": []}
    gtile = 0
    t0_br = np.full((NB, REG), -1, np.int64)  # global first tile of run
    for (b0, nbl) in cfg.supers:
        sup = {"b0": b0, "nb": nbl, "gt0": gtile, "g_calls": [], "blocks": []}
        scol = 0
        for r in range(REG):
            nt_r = int(tiles_br[b0:b0 + nbl, r].sum())
            if nt_r:
                sup["g_calls"].append(
                    {"region": r, "tile0": scol, "ntiles": nt_r})
            for bi in range(nbl):
                t0_br[b0 + bi, r] = gtile + scol
                scol += int(tiles_br[b0 + bi, r])
        sup["ntiles"] = scol
        for bi in range(nbl):
            b = b0 + bi
            runs = [(int(t0_br[b, r]), int(tiles_br[b, r]))
                    for r in range(REG) if tiles_br[b, r] > 0]
            sup["blocks"].append({"b": b, "runs": runs})
        gtile += scol
        meta["supers"].append(sup)
    NTILES = gtile
    meta["NTILES"] = NTILES

    # --- per-core index arrays -------------------------------------------
    idxg = np.zeros((C, 128, NTILES * 8), np.int16)
    rowrel = np.full((C, 128, NTILES), -1.0, np.float32)

    okey = (core * NB + eblk) * REG + ereg
    oorder = np.argsort(okey, kind="stable")
    ks = okey[oorder]
    starts = np.searchsorted(ks, np.arange(C * NB * REG))
    slot_in_run = np.empty(cfg.E, np.int64)
    slot_in_run[oorder] = np.arange(cfg.E) - starts[ks]

    gt0_e = t0_br[eblk, ereg]                 # first global tile of the run
    ek = gt0_e * 128 + slot_in_run            # global slot id
    etile = ek // 128
    epart = ek % 128

    oht = np.zeros((C, 128, NTILES * 128), np.uint8)
    ONE_FP8 = 0x38          # 1.0 in float8_e4m3
    for c in range(C):
        m = core == c
        rowrel[c, epart[m], etile[m]] = epos[m]
        kk = ek[m]
        idxg[c, kk % 16, (kk // 16)] = erow[m].astype(np.int16)
        oht[c, epos[m].astype(np.int64),
            etile[m] * 128 + epart[m]] = ONE_FP8
    for g in range(1, 8):
        idxg[:, g * 16:(g + 1) * 16, :] = idxg[:, 0:16, :]

    # --- parameters -------------------------------------------------------
    import ml_dtypes

    inv = np.empty((C, NSLOT), np.int64)
    have = np.zeros((C, NSLOT), bool)
    for c in range(C):
        nl = newlocal[c * NPC:(c + 1) * NPC]
        inv[c, nl] = np.arange(NPC)
        have[c, nl] = True
    featT = np.zeros((C, IN, NSLOT), ml_dtypes.bfloat16)
    for c in range(C):
        idx = inv[c][have[c]]
        featT[c][:, have[c]] = feat[c * NPC + idx].T.astype(ml_dtypes.bfloat16)

    a_r64 = np.asarray(a_r, np.float64)
    a_rs = np.where(np.abs(a_r64) < 1e-30, 1e-30, a_r64)
    Wp = (np.asarray(W, np.float64) * a_rs[None, :])        # [IN, OUT]
    wl = np.asarray(W, np.float64) @ np.asarray(a_l, np.float64)  # [IN]
    rhsk = []
    for k in range(cfg.KI):
        blk = np.concatenate(
            [Wp[k * 128:(k + 1) * 128],
             wl[k * 128:(k + 1) * 128, None]], axis=1)       # [128, 129]
        rhsk.append(np.ascontiguousarray(blk).astype(ml_dtypes.bfloat16))
    invarb = np.tile((1.0 / a_rs).astype(np.float32)[None, :], (128, 1))
    biasb = np.tile(np.asarray(bias, np.float32)[None, :], (128, 1))
    bsum = float(np.asarray(b_l, np.float64) + np.asarray(b_r, np.float64))
    bs_col = np.full((128, 1), bsum, np.float32)
    iota = np.tile(np.arange(128, dtype=np.float32)[None, :], (128, 1))
    iotab = iota.astype(ml_dtypes.bfloat16)

    in_maps = []
    for c in range(C):
        m = {
            "featT": featT[c], "biasb": biasb, "invarb": invarb,
            "bs": bs_col, "iotab": iotab,
            "idxg": idxg[c], "rowrel": rowrel[c],
            "oht": oht[c].view(ml_dtypes.float8_e4m3),
        }
        for k in range(cfg.KI):
            m[f"wk{k}"] = rhsk[k]
        in_maps.append(m)

    cfg.meta = meta

    def assemble(outs):
        full = np.empty((N, OUT), np.float32)
        for c in range(C):
            o = outs[c]["out"]
            nlc = newlocal[c * NPC:(c + 1) * NPC]
            full[c * NPC:(c + 1) * NPC] = o[nlc]
        return full

    return in_maps, assemble


def _build_program(cfg):
    C, IN, OUT = cfg.C, cfg.IN, cfg.OUT
    NB, NSLOT, KI, REG = cfg.NB, cfg.NSLOT, cfg.KI, cfg.REG
    RSLOT, RTB, RROWS = cfg.RSLOT, cfg.RTB, cfg.RROWS
    meta = cfg.meta
    NTILES = meta["NTILES"]
    NTMAX = max(s["ntiles"] for s in meta["supers"])
    FTC = RTB // 2 if RTB % 2 == 0 else RTB     # node tiles per featT chunk
    FCH = FTC * 128                             # featT load chunk columns

    nc = bacc.Bacc(None, num_swdge_queues=4, dynamic_dma_scratch_size=32768)
    featT = nc.declare_dram_parameter("featT", [IN, NSLOT], BF16, isOutput=False)
    wk = [nc.declare_dram_parameter(f"wk{k}", [128, OUT + 1], BF16,
                                    isOutput=False) for k in range(KI)]
    biasb = nc.declare_dram_parameter("biasb", [128, OUT], F32, isOutput=False)
    invarb = nc.declare_dram_parameter("invarb", [128, OUT], F32,
                                       isOutput=False)
    bs = nc.declare_dram_parameter("bs", [128, 1], F32, isOutput=False)
    iotab = nc.declare_dram_parameter("iotab", [128, 128], BF16, isOutput=False)
    idxg = nc.declare_dram_parameter("idxg", [128, NTILES * 8], I16,
                                     isOutput=False)
    rowrel = nc.declare_dram_parameter("rowrel", [128, NTILES], F32,
                                       isOutput=False)
    ohtp = nc.declare_dram_parameter("oht", [128, NTILES * 128], FP8,
                                     isOutput=False)
    outp = nc.declare_dram_parameter("out", [NSLOT, OUT], F32, isOutput=True)

    with tile.TileContext(nc) as tc:
        with (
            tc.tile_pool(name="dram", bufs=1, space="DRAM") as dram,
            tc.tile_pool(name="consts", bufs=1) as cp,
            tc.tile_pool(name="nfeat", bufs=1) as nfp,
            tc.tile_pool(name="naug", bufs=2) as nap,
            tc.tile_pool(name="npsum", bufs=2, space="PSUM") as npp,
            tc.tile_pool(name="eidx", bufs=2) as eip,
            tc.tile_pool(name="eoht", bufs=2) as ehp,
            tc.tile_pool(name="f1ps", bufs=2, space="PSUM") as f1pp,
            tc.tile_pool(name="egath", bufs=3) as egp,
            tc.tile_pool(name="ecol", bufs=2) as ecp,
            tc.tile_pool(name="ewt", bufs=4) as ewp,
            tc.tile_pool(name="epsum", bufs=2, space="PSUM") as epp,
            tc.tile_pool(name="eout", bufs=3) as eop,
        ):
            agin = [dram.tile([RSLOT, OUT], BF16, name=f"agin{r}")
                    for r in range(REG)]
            tabr = [dram.tile([RROWS, OUT], BF16, name=f"tabr{r}",
                              addr_space="Shared") for r in range(REG)]

            # ---- constants ----
            wk_sb = []
            for k in range(KI):
                w_t = cp.tile([128, OUT + 1], BF16, name=f"wksb{k}")
                nc.sync.dma_start(w_t[:], wk[k][:])
                wk_sb.append(w_t)
            biasb_sb = cp.tile([128, OUT], F32)
            nc.sync.dma_start(biasb_sb[:], biasb[:])
            invarb_sb = cp.tile([128, OUT], F32)
            nc.sync.dma_start(invarb_sb[:], invarb[:])
            bs_sb = cp.tile([128, 1], F32)
            nc.sync.dma_start(bs_sb[:], bs[:])
            iota_sb = cp.tile([128, 128], BF16)
            nc.sync.dma_start(iota_sb[:], iotab[:])
            ones_sb = cp.tile([128, 1], BF16)
            nc.vector.memset(ones_sb[:], 1.0)
            f1acc = cp.tile([128, 128], F32)
            nc.vector.memset(f1acc[:], 0.0)
            f1bbq = []

            # ---- node phase (region-major) ----
            for r in range(REG):
                fts = {}
                for k in range(KI):
                    for h in range(RTB // FTC):
                        ft = nfp.tile([128, FCH], BF16, name=f"ft{k}{h}")
                        c0 = r * RSLOT + h * FCH
                        nc.sync.dma_start(
                            ft[:], featT[k * 128:(k + 1) * 128, c0:c0 + FCH])
                        fts[(k, h)] = ft
                aug = nap.tile([128, RSLOT], BF16, name="aug")
                for ntl in range(RTB):
                    nt = r * RTB + ntl
                    ps = npp.tile([128, OUT + 1], F32)
                    for k in range(KI):
                        h, off = divmod(ntl * 128, FCH)
                        ft = fts[(k, h)]
                        nc.tensor.matmul(ps[:], lhsT=ft[:, off:off + 128],
                                         rhs=wk_sb[k][:],
                                         start=(k == 0), stop=(k == KI - 1))
                    nc.vector.tensor_copy(aug[:, ntl * 128:(ntl + 1) * 128],
                                          ps[:, 0:128])
                    nc.vector.tensor_copy(f1acc[:, nt:nt + 1],
                                          ps[:, 128:129])
                f1q = cp.tile([128, RTB], BF16, name=f"f1bbq{r}")
                nc.vector.tensor_scalar(
                    out=f1q[:], in0=f1acc[:, r * RTB:(r + 1) * RTB],
                    scalar1=bs_sb[:], scalar2=None, op0=ALU.add)
                f1bbq.append(f1q)
                nc.sync.dma_start(
                    agin[r][:, :].rearrange("(t p) o -> p t o", p=128),
                    aug[:].rearrange("p (t o) -> p t o", o=OUT))
                nc.gpsimd.collective_compute(
                    "AllGather", ALU.bypass,
                    replica_groups=[list(range(C))],
                    ins=[agin[r].opt()],
                    outs=[tabr[r].opt()],
                )


            # ---- edge phase ----
            self_qn = [0]
            tix = [0]     # global tile counter for DVE/ACT f2 split
            for sup in meta["supers"]:
                ntiles = sup["ntiles"]
                gt0 = sup["gt0"]
                ixg = eip.tile([128, NTMAX * 8], I16, name="ixg")
                nc.sync.dma_start(ixg[:, 0:ntiles * 8],
                                  idxg[:, gt0 * 8:(gt0 + ntiles) * 8])
                rr_sb = eip.tile([128, NTMAX], F32, name="rr_sb")
                nc.sync.dma_start(rr_sb[:, 0:ntiles],
                                  rowrel[:, gt0:gt0 + ntiles])
                oht_sb = ehp.tile([128, NTMAX * 128], FP8, name="oht_sb")
                nc.sync.dma_start(oht_sb[:, 0:ntiles * 128],
                                  ohtp[:, gt0 * 128:(gt0 + ntiles) * 128])

                G = egp.tile([128, NTMAX * 128], BF16, name="G")
                G3 = G[:].rearrange("p (t c) -> p t c", c=128)
                for g in sup["g_calls"]:
                    r = g["region"]
                    for ct0 in range(0, g["ntiles"], GCHUNK):
                        cn = min(GCHUNK, g["ntiles"] - ct0)
                        lt0 = g["tile0"] + ct0
                        nc.gpsimd.dma_gather(
                            out_ap=G[:, lt0 * 128:(lt0 + cn) * 128]
                            .rearrange("p (t e) -> p t e", e=128),
                            in_ap=tabr[r][:],
                            idxs_ap=ixg[:, lt0 * 8:(lt0 + cn) * 8],
                            num_idxs=cn * 128,
                            num_idxs_reg=cn * 128,
                            elem_size=128,
                            queue_num=self_qn[0] % 4,
                        )
                        self_qn[0] += 1

                # f2 per edge = row-sum of the gathered (a_r-scaled) row;
                # split between DVE reduce and ACT copy-accum
                F2 = ecp.tile([128, NTMAX], F32, name="F2")
                for t in range(ntiles):
                    tix[0] += 1
                    if (tix[0] % 10) < int(ACT_F2_FRAC * 10):
                        dmy = ecp.tile([128, 128], BF16, name="dmy")
                        nc.scalar.activation(dmy[:], G3[:, t, :], AF.Copy,
                                             accum_out=F2[:, t:t + 1])
                    else:
                        nc.vector.tensor_reduce(
                            out=F2[:, t:t + 1], in_=G3[:, t, :],
                            axis=mybir.AxisListType.X, op=ALU.add)

                # f1e per edge on PE: psf[:, j] = OHT_t^T @ f1col(b)
                TT = ecp.tile([128, NTMAX], F32, name="TT")
                BPQ = cfg.BPQ
                for blk in sup["blocks"]:
                    b = blk["b"]
                    q = b // BPQ
                    psf = f1pp.tile([128, 32], F32, name="psf")
                    j = 0
                    spans = []
                    for (t0, nt) in blk["runs"]:
                        spans.append((j, t0 - gt0, nt))
                        for t in range(t0 - gt0, t0 - gt0 + nt):
                            nc.tensor.matmul(
                                psf[:, j:j + 1],
                                lhsT=oht_sb[:, t * 128:(t + 1) * 128],
                                rhs=f1bbq[q][:, (b % BPQ):(b % BPQ) + 1],
                                start=True, stop=True)
                            j += 1
                    # t = f1e + f2 written straight into the super's TT cols
                    for (j0, tl0, nt) in spans:
                        nc.vector.tensor_tensor(
                            out=TT[:, tl0:tl0 + nt],
                            in0=psf[:, j0:j0 + nt],
                            in1=F2[:, tl0:tl0 + nt], op=ALU.add)

                # w = exp(0.6 t + 0.4 |t|)
                AA = ecp.tile([128, NTMAX], F32, name="AA")
                nc.scalar.activation(AA[:, 0:ntiles], TT[:, 0:ntiles],
                                     AF.Abs, scale=0.4)
                ZZ = ecp.tile([128, NTMAX], F32, name="ZZ")
                nc.vector.scalar_tensor_tensor(
                    out=ZZ[:, 0:ntiles], in0=TT[:, 0:ntiles], scalar=0.6,
                    in1=AA[:, 0:ntiles], op0=ALU.mult, op1=ALU.add)
                WW = ecp.tile([128, NTMAX], F32, name="WW")
                nc.scalar.activation(WW[:, 0:ntiles], ZZ[:, 0:ntiles], AF.Exp)

                for blk in sup["blocks"]:
                    b = blk["b"]
                    tl = []
                    for (t0, nt) in blk["runs"]:
                        tl.extend(range(t0 - gt0, t0 - gt0 + nt))
                    if not tl:
                        continue
                    ps = epp.tile([128, OUT], F32, name="bps")
                    ps2 = epp.tile([128, 1], F32, name="bps2")
                    for j, t in enumerate(tl):
                        wt = ewp.tile([128, 128], BF16, name="wt")
                        nc.vector.tensor_scalar(
                            out=wt[:], in0=iota_sb[:],
                            scalar1=rr_sb[:, t:t + 1],
                            scalar2=WW[:, t:t + 1],
                            op0=ALU.is_equal, op1=ALU.mult)
                        first = j == 0
                        last = j == len(tl) - 1
                        nc.tensor.matmul(
                            ps[:], lhsT=wt[:],
                            rhs=G3[:, t, :],
                            start=first, stop=last)
                        nc.tensor.matmul(
                            ps2[:], lhsT=wt[:],
                            rhs=ones_sb[:],
                            start=first, stop=last)
                    sden = eop.tile([128, 1], F32, name="sden")
                    nc.vector.tensor_scalar(out=sden[:],
                                            in0=ps2[:],
                                            scalar1=1e-9, scalar2=None,
                                            op0=ALU.add)
                    rcp = eop.tile([128, 1], F32, name="rcp")
                    nc.vector.reciprocal(rcp[:], sden[:])
                    xx1 = eop.tile([128, OUT], F32, name="xx1")
                    nc.vector.scalar_tensor_tensor(
                        out=xx1[:], in0=ps[:], scalar=rcp[:],
                        in1=invarb_sb[:], op0=ALU.mult, op1=ALU.mult)
                    xx = eop.tile([128, OUT], F32, name="xx")
                    nc.vector.tensor_tensor(out=xx[:], in0=xx1[:],
                                            in1=biasb_sb[:], op=ALU.add)
                    ee = eop.tile([128, OUT], F32, name="ee")
                    nc.scalar.activation(ee[:], xx[:], AF.Exp)
                    ov = eop.tile([128, OUT], F32, name="ov")
                    nc.vector.tensor_scalar(out=ov[:], in0=ee[:],
                                            scalar1=-1.0, scalar2=None,
                                            op0=ALU.add)
                    mk = eop.tile([128, OUT], U8, name="mk")
                    nc.vector.tensor_scalar(out=mk[:], in0=xx[:],
                                            scalar1=0.0, scalar2=None,
                                            op0=ALU.is_gt)
                    nc.vector.copy_predicated(ov[:], mk[:], xx[:])
                    nc.sync.dma_start(outp[b * 128:(b + 1) * 128, :], ov[:])

    nc.finalize()
    return nc


def _run(cfg, inputs, trace=False):
    in_maps, assemble = _prep_host(
        cfg,
        np.asarray(inputs["feat"], np.float32),
        np.asarray(inputs["W"], np.float32),
        np.asarray(inputs["a_l"], np.float32),
        np.asarray(inputs["b_l"], np.float32),
        np.asarray(inputs["a_r"], np.float32),
        np.asarray(inputs["b_r"], np.float32),
        np.asarray(inputs["bias"], np.float32),
        np.asarray(inputs["row"]),
        np.asarray(inputs["col"]),
    )
    nc = _build_program(cfg)
    res = run_bass_kernel_spmd(nc, in_maps, list(range(cfg.C)), trace=trace)
    return assemble(res.results), res


def kernel(**inputs):
    feat = np.asarray(inputs["feat"])
    row = np.asarray(inputs["row"])
    cfg = _Cfg(N=feat.shape[0], E=row.shape[0], IN=feat.shape[1],
               OUT=np.asarray(inputs["W"]).shape[1], C=8)
    out, _ = _run(cfg, inputs, trace=False)
    return out
